# revision 1
# baseline (speedup 1.0000x reference)
"""Trainium2 Bass kernel for ConvGPTAttention (dense transformer attention block).

Sharding: tensor-parallel by head groups across 8 NeuronCores.
Core j owns q heads {2j, 2j+1} and kv head j (GQA maps q head h -> kv head h//2,
so each core's attention is fully local). Wqkv is column-sharded, Wo is
row-sharded; the 8 partial o_proj outputs are summed on the host (the
"all-reduce" of RowParallelLinear, done at unshard time).

Per-core pipeline (all matmuls in float32r = full-rate ~fp32):
  Phase A: qkv = X @ Wqkv_shard in token-major layout [t, 512] (per-hi-tile
           DMA streaming of X^T panels); fused per-head RMSNorm
           (square/reduce/sqrt/recip) + neox RoPE via host-computed
           coefficient tables (norm weight folded in, derived from
           `positions`); PE-transpose of q/k to [d, t] layout.
  Phase B: causal attention per (q head, 512-col t-block) in S^T layout at
           s-block (128-token) granularity: S^T = kT.T @ qT, additive causal
           mask on diagonal s-blocks (DVE), exp via ScalarE (softmax scale
           folded into the activation), attn^T accumulated as v^T @ expS^T
           in PSUM with av/sum-exp matmuls lagging two s-blocks behind S^T
           in the PE stream; softmax denominator via ones-vector matmul,
           reciprocal on DVE, broadcast across partitions on GpSimd.
           o_proj units (fp16 output partials) woven into the next t-block's
           attention stream.
"""

import numpy as np
from contextlib import ExitStack

import concourse.bacc as bacc
import concourse.mybir as mybir
import concourse.tile as tile
from concourse.bass_utils import run_bass_kernel_spmd

P = 128
T = 2048
H = 2048
N_HEADS = 16
N_KV = 8
HD = 128
EPS = 1e-6
THETA = 10000.0
SCALE = HD ** -0.5
NEG = -60000.0  # additive mask (fp16-safe); SCALE*NEG = -5303 -> exp == 0.0

F32 = mybir.dt.float32
F32R = mybir.dt.float32r
AF = mybir.ActivationFunctionType
ALU = mybir.AluOpType

N_CORES = 8
N_TT = 16        # t-tiles of 128 tokens
N_TB = 4         # t-blocks of 512 tokens (attention rhs width)


def _build_nc():
    nc = bacc.Bacc("TRN2", target_bir_lowering=False, debug=False)

    xt = nc.dram_tensor("xt", [H, T], F32R, kind="ExternalInput")
    w = nc.dram_tensor("w", [H, 512], F32R, kind="ExternalInput")
    wo = nc.dram_tensor("wo", [256, H], F32R, kind="ExternalInput")
    ab = nc.dram_tensor("ab", [T, 4, 3, 64], mybir.dt.float16, kind="ExternalInput")
    maska = nc.dram_tensor("maska", [P, 2, 512], mybir.dt.float16, kind="ExternalInput")
    maskb = nc.dram_tensor("maskb", [P, 2, 512], mybir.dt.float16, kind="ExternalInput")
    ident = nc.dram_tensor("ident", [P, P], F32, kind="ExternalInput")
    ones = nc.dram_tensor("ones", [P, 1], F32R, kind="ExternalInput")
    out = nc.dram_tensor("out", [T, H], mybir.dt.float16, kind="ExternalOutput")

    with ExitStack() as top:
        tc = top.enter_context(tile.TileContext(nc))
        pers = top.enter_context(tc.tile_pool(name="pers", bufs=1))

        maska_sb = pers.tile([P, 2, 512], mybir.dt.float16, tag="maska")
        maskb_sb = pers.tile([P, 2, 512], mybir.dt.float16, tag="maskb")
        ident_sb = pers.tile([P, P], F32, tag="ident")
        nc.sync.dma_start(ident_sb[:], ident[:])
        ones_sb = pers.tile([P, 1], F32R, tag="ones")
        nc.sync.dma_start(ones_sb[:], ones[:])
        eps_sb = pers.tile([P, 1], F32, tag="eps")
        nc.vector.memset(eps_sb[:], EPS)

        # persistent activations
        qT = pers.tile([P, 3, T], F32R, tag="qT")      # [d, (q0,q1,k), t]
        v_tok = pers.tile([P, N_TT, P], F32R, tag="v")  # [t_in, tt, d]

        # ---------------- Phase A: QKV + norm + rope + transpose ------------
        with ExitStack() as pa_ctx:
            wp = pa_ctx.enter_context(tc.tile_pool(name="wp", bufs=1))
            xtp = pa_ctx.enter_context(tc.tile_pool(name="xtp", bufs=2))
            pa = pa_ctx.enter_context(tc.tile_pool(name="pa", bufs=2))
            psa = pa_ctx.enter_context(tc.tile_pool(name="psa", bufs=3, space="PSUM"))
            pst = pa_ctx.enter_context(tc.tile_pool(name="pst", bufs=1, space="PSUM"))

            w_sb = wp.tile([P, 16, 512], F32R, tag="w")

            for tsb in range(4):          # 512-token superblocks (XT panels)
                xt_sb = xtp.tile([P, 16, 512], F32R, tag="xt")
                ab_sb = xtp.tile([P, 4, 4, 3, 64], mybir.dt.float16, tag="ab")
                # per-hi DMAs so the first matmuls start after ~512KB, not 8MB;
                # rope tables slip in mid-panel so rope never starves
                for hi in range(16):
                    if tsb == 0:
                        nc.sync.dma_start(
                            w_sb[:, hi, :], w[hi * 128:(hi + 1) * 128, :]
                        )
                    nc.sync.dma_start(
                        xt_sb[:, hi, :],
                        xt[hi * 128:(hi + 1) * 128, tsb * 512:(tsb + 1) * 512],
                    )
                    if hi % 4 == 3:
                        tt_l = hi // 4
                        t0 = tsb * 512 + tt_l * 128
                        nc.sync.dma_start(ab_sb[:, tt_l], ab[t0:t0 + 128])
                if tsb == 2:
                    nc.sync.dma_start(maska_sb[:], maska[:])
                elif tsb == 3:
                    nc.sync.dma_start(maskb_sb[:], maskb[:])
                def post_process(tg, tgl, ps_a, ab_sb=ab_sb):
                    # views on the psum group: [P, 2(ttl), 4(head), 128(d)]
                    ps_r = ps_a.rearrange("p g (h d) -> p g h d", h=4)

                    # v: straight copy (cast to fp32r), token-major
                    nc.scalar.copy(v_tok[:, 2 * tg:2 * tg + 2, :], ps_r[:, :, 3, :])

                    # RMS stats on raw q/k: sumsq -> sqrt(mean+eps) -> 1/x
                    sq = pa.tile([P, 2, 3, 128], F32, tag="sq")
                    nc.scalar.activation(sq[:], ps_r[:, :, 0:3, :], AF.Square)
                    ss = pa.tile([P, 2, 3], F32, tag="ss")
                    nc.vector.tensor_reduce(
                        ss[:], sq[:], axis=mybir.AxisListType.X, op=ALU.add
                    )
                    sr = pa.tile([P, 2, 3], F32, tag="sr")
                    nc.scalar.activation(
                        sr[:], ss[:], AF.Sqrt, scale=1.0 / HD, bias=eps_sb[:]
                    )
                    s_inv = pa.tile([P, 2, 3], F32, tag="si")
                    nc.vector.reciprocal(s_inv[:], sr[:])

                    # rope (tables have norm weight folded in, heads packed
                    # (q0, q1, k) along the table's head dim):
                    # out1 = x1*a1 - x2*b1 ; out2 = x2*a2 + x1*b2
                    qkn = pa.tile([P, 2, 3, 128], F32, tag="qkn")
                    x1 = ps_r[:, :, 0:3, 0:64]
                    x2 = ps_r[:, :, 0:3, 64:128]
                    abg = ab_sb[:, 2 * tgl:2 * tgl + 2]    # [P, 2, 4, 3, 64]
                    m1 = pa.tile([P, 2, 3, 64], F32, tag="m1")
                    m2 = pa.tile([P, 2, 3, 64], F32, tag="m2")
                    nc.vector.tensor_mul(m1[:], x1, abg[:, :, 0])
                    nc.vector.tensor_mul(m2[:], x2, abg[:, :, 1])
                    nc.vector.tensor_sub(qkn[:, :, :, 0:64], m1[:], m2[:])
                    nc.vector.tensor_mul(m1[:], x2, abg[:, :, 2])
                    nc.vector.tensor_mul(m2[:], x1, abg[:, :, 3])
                    nc.vector.tensor_add(qkn[:, :, :, 64:128], m1[:], m2[:])

                    # apply 1/rms (per token+head, broadcast over d)
                    nc.vector.tensor_mul(
                        qkn[:],
                        qkn[:],
                        s_inv[:, :, :, None].to_broadcast((P, 2, 3, 128)),
                    )

                    # transpose q0/q1/k to [d, t]
                    for ttl in range(2):
                        tt = 2 * tg + ttl
                        ps_t = pst.tile([P, 3, P], F32, tag="pst")
                        for h in range(3):
                            nc.tensor.transpose(
                                ps_t[:, h, :], qkn[:, ttl, h, :], ident_sb[:]
                            )
                        nc.scalar.copy(
                            qT[:, :, tt * 128:(tt + 1) * 128], ps_t[:]
                        )

                if tsb == 0:
                    # warmup: hi-major across all 4 psum chains so each
                    # arriving (W[hi], X[hi]) DMA pair feeds 4 matmuls
                    pair = [
                        psa.tile([P, 2, 512], F32, tag="psa", name=f"psa_w{i}")
                        for i in range(2)
                    ]
                    for hi in range(16):
                        for tgl in range(2):
                            for ttl in range(2):
                                nc.tensor.matmul(
                                    pair[tgl][:, ttl, :],
                                    xt_sb[:, hi, tgl * 256 + ttl * 128:
                                          tgl * 256 + (ttl + 1) * 128],
                                    w_sb[:, hi, :],
                                    start=(hi == 0),
                                    stop=(hi == 15),
                                )
                    for tgl in range(2):
                        post_process(tgl, tgl, pair[tgl])
                    continue
                for tgl in range(2):  # groups of 2 t-tiles (256 tokens)
                    ps_a = psa.tile([P, 2, 512], F32, tag="psa")
                    for ttl in range(2):
                        for hi in range(16):
                            nc.tensor.matmul(
                                ps_a[:, ttl, :],
                                xt_sb[:, hi, tgl * 256 + ttl * 128:
                                      tgl * 256 + (ttl + 1) * 128],
                                w_sb[:, hi, :],
                                start=(hi == 0),
                                stop=(hi == 15),
                            )
                    post_process(tsb * 2 + tgl, tgl, ps_a)

        # ---------------- Phase B: attention + o_proj -----------------------
        with ExitStack() as pb_ctx:
            pb = pb_ctx.enter_context(tc.tile_pool(name="pb", bufs=1))
            expp = pb_ctx.enter_context(tc.tile_pool(name="expp", bufs=3))
            nrm = pb_ctx.enter_context(tc.tile_pool(name="nrm", bufs=2))
            outp = pb_ctx.enter_context(tc.tile_pool(name="outp", bufs=6))
            pss = pb_ctx.enter_context(tc.tile_pool(name="pss", bufs=3, space="PSUM"))
            psat = pb_ctx.enter_context(tc.tile_pool(name="psat", bufs=2, space="PSUM"))
            psse = pb_ctx.enter_context(tc.tile_pool(name="psse", bufs=1, space="PSUM"))
            pso = pb_ctx.enter_context(tc.tile_pool(name="pso", bufs=2, space="PSUM"))

            wo_sb = pb.tile([P, 2, H], F32R, tag="wo")
            nc.sync.dma_start(wo_sb[:], wo.rearrange("(do p) h -> p do h", p=P))
            attn_sb = pb.tile([P, 2, T], F32R, tag="attn")  # [d, qh, t] normalized

            def emit_oproj_unit(tt, hb):
                ps_o = pso.tile([P, 512], F32, tag="o")
                for hh in range(2):
                    nc.tensor.matmul(
                        ps_o[:],
                        attn_sb[:, hh, tt * 128:(tt + 1) * 128],
                        wo_sb[:, hh, hb * 512:(hb + 1) * 512],
                        start=(hh == 0),
                        stop=(hh == 1),
                    )
                o_sb = outp.tile([P, 512], mybir.dt.float16, tag="osb")
                if hb % 2 == 0:
                    nc.scalar.copy(o_sb[:], ps_o[:])
                else:
                    nc.vector.tensor_copy(o_sb[:], ps_o[:])
                nc.sync.dma_start(
                    out[tt * 128:(tt + 1) * 128, hb * 512:(hb + 1) * 512],
                    o_sb[:],
                )

            # o_proj units of t-block tbo, woven into the next attention
            # t-block's PE stream
            oproj_queue = []

            def queue_oproj(tbo):
                for ttl in range(4):
                    for hb in range(4):
                        oproj_queue.append((4 * tbo + ttl, hb))

            for tb in range(N_TB):
                qrhs0 = tb * 512
                for qh in range(2):
                    ps_at = psat.tile([P, 512], F32, tag="at")
                    ps_se = psse.tile([1, 512], F32, tag="se")
                    nsb = 4 * (tb + 1)    # s-blocks of 128 tokens
                    # diagonal (masked) s-blocks first (their longer
                    # S^T -> mask -> exp chain pipelines over later blocks);
                    # av/se lag two blocks behind S^T in the PE stream so the
                    # PE never waits on exp.
                    diag = list(range(4 * tb, 4 * tb + 4))
                    rest = list(range(4 * tb))
                    sb_order = []
                    for i in range(max(len(diag), len(rest))):
                        if i < len(diag):
                            sb_order.append(diag[i])
                        if i < len(rest):
                            sb_order.append(rest[i])

                    def emit_av(sb, si, o):
                        es = es_tiles[sb]
                        nc.tensor.matmul(
                            ps_at[:, o:512],
                            v_tok[:, sb, :],
                            es[:, o:512],
                            start=(si == 0),
                            stop=(si == nsb - 1),
                            skip_group_check=True,
                        )
                        nc.tensor.matmul(
                            ps_se[:, o:512],
                            ones_sb[:],
                            es[:, o:512],
                            start=(si == 0),
                            stop=(si == nsb - 1),
                            skip_group_check=True,
                        )

                    es_tiles = {}
                    pending = []
                    for si, sb in enumerate(sb_order):
                        off = sb - 4 * tb
                        # diagonal blocks: columns left of the triangle are
                        # fully causally masked -- skip them outright
                        o = 128 * off if 0 <= off < 4 else 0
                        ps_s = pss.tile([P, 512], F32, tag="st")
                        nc.tensor.matmul(
                            ps_s[:, o:512],
                            qT[:, 2, sb * 128:(sb + 1) * 128],
                            qT[:, qh, qrhs0 + o:qrhs0 + 512],
                            start=True,
                            stop=True,
                        )
                        if 0 <= off < 2:
                            nc.vector.tensor_add(
                                ps_s[:, o:o + 128], ps_s[:, o:o + 128],
                                maska_sb[:, off, o:o + 128],
                            )
                        elif 2 <= off < 4:
                            nc.vector.tensor_add(
                                ps_s[:, o:o + 128], ps_s[:, o:o + 128],
                                maskb_sb[:, off - 2, o:o + 128],
                            )
                        es = expp.tile([P, 512], F32R, tag="es")
                        es_tiles[sb] = es
                        nc.scalar.activation(
                            es[:, o:512], ps_s[:, o:512], AF.Exp, scale=SCALE
                        )
                        pending.append((sb, si, o))
                        if len(pending) > 2:
                            emit_av(*pending.pop(0))
                        if oproj_queue and (si % 2 == 0 or len(oproj_queue) > 8):
                            emit_oproj_unit(*oproj_queue.pop(0))
                    for item in pending:
                        emit_av(*item)
                    # softmax denominator -> broadcast -> normalize
                    inv_sb = nrm.tile([1, 512], F32, tag="inv")
                    nc.vector.reciprocal(inv_sb[:], ps_se[:])
                    bc = nrm.tile([P, 512], F32, tag="bc")
                    nc.gpsimd.partition_broadcast(bc[:], inv_sb[0:1, :])
                    if tb == N_TB - 1:
                        for ttn in range(4):
                            sl = slice(ttn * 128, (ttn + 1) * 128)
                            nc.vector.tensor_mul(
                                attn_sb[:, qh, qrhs0 + ttn * 128:
                                        qrhs0 + (ttn + 1) * 128],
                                ps_at[:, sl], bc[:, sl],
                            )
                    else:
                        nc.vector.tensor_mul(
                            attn_sb[:, qh, qrhs0:qrhs0 + 512], ps_at[:], bc[:]
                        )
                while oproj_queue:
                    emit_oproj_unit(*oproj_queue.pop(0))
                queue_oproj(tb)
            while oproj_queue:
                emit_oproj_unit(*oproj_queue.pop(0))

    nc.compile()
    return nc


_NC_CACHE = {}


def _get_nc():
    if "nc" not in _NC_CACHE:
        _NC_CACHE["nc"] = _build_nc()
    return _NC_CACHE["nc"]


def kernel(positions, hidden_states, Wqkv, Wo, q_norm_w, k_norm_w):
    positions = np.asarray(positions)
    out_dtype = np.asarray(hidden_states).dtype
    hs = np.asarray(hidden_states, dtype=np.float32)
    Wqkv = np.asarray(Wqkv, dtype=np.float32)
    Wo = np.asarray(Wo, dtype=np.float32)
    qw = np.asarray(q_norm_w, dtype=np.float32)
    kw = np.asarray(k_norm_w, dtype=np.float32)

    # ----- host-side input prep -----
    xt = np.ascontiguousarray(hs.T)

    inv_freq = (1.0 / (THETA ** (np.arange(0, HD, 2, dtype=np.float32) / HD))).astype(
        np.float32
    )
    freqs = positions.astype(np.float32)[:, None] * inv_freq[None, :]  # [T, 64]
    cos = np.cos(freqs).astype(np.float32)
    sin = np.sin(freqs).astype(np.float32)

    def ab_tables(wvec):
        a1 = cos * wvec[None, :64]
        b1 = sin * wvec[None, 64:]
        a2 = cos * wvec[None, 64:]
        b2 = sin * wvec[None, :64]
        return np.stack([a1, b1, a2, b2], axis=1)  # [T, 4, 64]

    abq = ab_tables(qw)
    abk = ab_tables(kw)
    # combined per-head tables, head order (q0, q1, k)
    ab = np.ascontiguousarray(
        np.stack([abq, abq, abk], axis=2), dtype=np.float16
    )  # [T, 4, 3, 64]

    # causal masks for the two diagonal 256-row groups of each 512-col t-block
    # (positions is monotonically increasing per the spec, so causality is the
    # standard band structure)
    t_in = np.arange(512)
    s_in = np.arange(128)

    def mk_mask(offsets):
        m = np.empty((P, 2, 512), dtype=np.float16)
        for j, o in enumerate(offsets):
            m[:, j, :] = np.where(
                (o + s_in)[:, None] <= t_in[None, :], 0.0, NEG
            )
        return m

    maska = mk_mask((0, 128))
    maskb = mk_mask((256, 384))
    ident = np.eye(P, dtype=np.float32)
    ones = np.ones((P, 1), dtype=np.float32)

    q_size = N_HEADS * HD
    kv_size = N_KV * HD
    in_maps = []
    for j in range(N_CORES):
        qs = slice(2 * j * HD, (2 * j + 2) * HD)
        ks = slice(q_size + j * HD, q_size + (j + 1) * HD)
        vs = slice(q_size + kv_size + j * HD, q_size + kv_size + (j + 1) * HD)
        wj = np.ascontiguousarray(
            np.concatenate([Wqkv[:, qs], Wqkv[:, ks], Wqkv[:, vs]], axis=1)
        )
        woj = np.ascontiguousarray(Wo[qs, :])
        in_maps.append(
            {
                "xt": xt,
                "w": wj,
                "wo": woj,
                "ab": ab,
                "maska": maska,
                "maskb": maskb,
                "ident": ident,
                "ones": ones,
            }
        )

    nc = _get_nc()
    res = run_bass_kernel_spmd(nc, in_maps, core_ids=list(range(N_CORES)))

    acc = res.results[0]["out"].astype(np.float32)
    for j in range(1, N_CORES):
        acc += res.results[j]["out"].astype(np.float32)
    return acc.astype(out_dtype, copy=False)



# revision 6
# speedup vs baseline: 1.2156x; 1.2156x over previous
"""Trainium2 Bass kernel for ConvGPTAttention (dense transformer attention block).

Sharding: tensor-parallel by head groups across 8 NeuronCores.
Core j owns q heads {2j, 2j+1} and kv head j (GQA maps q head h -> kv head h//2,
so each core's attention is fully local). Wqkv is column-sharded, Wo is
row-sharded; the 8 partial o_proj outputs are summed on the host (the
"all-reduce" of RowParallelLinear, done at unshard time).

Numerics/dtype plan (validated against the reference in fp64 emulation):
  - QKV GEMM: fp8e4m3 DoubleRow matmuls with full error compensation:
    qkv = (X_hi + X_lo) @ W_hi + X_hi @ W_lo  (X_lo/W_lo are fp8 quantization
    residuals). W pre-scaled by 64 on the host; RMSNorm is scale-invariant.
  - q/k post-norm+rope stored fp16; attention scores in fp16 matmuls.
  - Causal masking is done on the PE: an extra matmul accumulates a
    [128,128] NEG upper-triangle block (maskT^T @ I) into the diagonal
    score blocks, so no DVE/Pool masking pass is needed.
  - softmax: exp on ScalarE with scale=1/sqrt(128), bias=-2ln2 (keeps fp8
    range safe); es stored fp8; denominator via a [128,2,1]=32.0 DoubleRow
    ones-matmul sharing the es stream; AV via fp8 DoubleRow over s-block
    pairs (v stored fp8 at 32x scale).
  - o_proj in fp16; partial outputs written fp16, summed on host in fp32.
"""

import numpy as np
import ml_dtypes
from contextlib import ExitStack

import concourse.bacc as bacc
import concourse.mybir as mybir
import concourse.tile as tile
from concourse.bass_utils import run_bass_kernel_spmd

P = 128
T = 2048
H = 2048
N_HEADS = 16
N_KV = 8
HD = 128
EPS = 1e-6
THETA = 10000.0
SCALE = HD ** -0.5
NEG = -60000.0
EXP_BIAS = -2.0 * float(np.log(2.0))

F32 = mybir.dt.float32
F16 = mybir.dt.float16
F8 = mybir.dt.float8e4
AF = mybir.ActivationFunctionType
ALU = mybir.AluOpType
DR = mybir.MatmulPerfMode.DoubleRow

NP_F8 = ml_dtypes.float8_e4m3

N_CORES = 8
N_TT = 16        # t-tiles of 128 tokens
N_TB = 4         # t-blocks of 512 tokens


def _build_nc():
    nc = bacc.Bacc("TRN2", target_bir_lowering=False, debug=False)

    xhl = nc.dram_tensor("xhl", [P, 4, 16, 2, 512], F8, kind="ExternalInput")
    whi = nc.dram_tensor("whi", [P, 16, 512], F8, kind="ExternalInput")
    wlo = nc.dram_tensor("wlo", [P, 16, 512], F8, kind="ExternalInput")
    wo = nc.dram_tensor("wo", [256, H], F16, kind="ExternalInput")
    ab = nc.dram_tensor("ab", [T, 4, 3, 64], F16, kind="ExternalInput")
    maskt = nc.dram_tensor("maskt", [P, P], F16, kind="ExternalInput")
    ident = nc.dram_tensor("ident", [P, P], F16, kind="ExternalInput")
    ones = nc.dram_tensor("ones", [P, 2], F8, kind="ExternalInput")
    out = nc.dram_tensor("out", [T, H], F16, kind="ExternalOutput")

    with ExitStack() as top:
        tc = top.enter_context(tile.TileContext(nc))
        pers = top.enter_context(tc.tile_pool(name="pers", bufs=1))

        ident_sb = pers.tile([P, P], F16, tag="ident")
        nc.sync.dma_start(ident_sb[:], ident[:])
        maskt_sb = pers.tile([P, P], F16, tag="maskt")
        nc.sync.dma_start(maskt_sb[:], maskt[:])
        ones_sb = pers.tile([P, 2, 1], F8, tag="ones")
        nc.sync.dma_start(ones_sb[:], ones.rearrange("p (a b) -> p a b", b=1))
        wo_sb = pers.tile([P, 2, H], F16, tag="wo")
        nc.sync.dma_start(wo_sb[:], wo.rearrange("(q p) h -> p q h", p=P))
        eps_sb = pers.tile([P, 1], F32, tag="eps")
        nc.vector.memset(eps_sb[:], 4096.0 * EPS)
        ebias_sb = pers.tile([P, 1], F32, tag="ebias")
        nc.vector.memset(ebias_sb[:], EXP_BIAS)

        # persistent activations
        qkT = pers.tile([P, 3, T], F16, tag="qkT")      # [d, (q0,q1,k), t]
        v8 = pers.tile([P, N_TT, P], F8, tag="v8")      # [s_in, tt, d] (32x)
        attn16 = pers.tile([P, 2, T], F16, tag="attn")  # [d, qh, t] normalized

        # ---------------- Phase A: QKV + norm + rope + transpose ------------
        with ExitStack() as pa_ctx:
            wp = pa_ctx.enter_context(tc.tile_pool(name="wp", bufs=1))
            xp = pa_ctx.enter_context(tc.tile_pool(name="xp", bufs=2))
            pa = pa_ctx.enter_context(tc.tile_pool(name="pa", bufs=2))
            psa = pa_ctx.enter_context(tc.tile_pool(name="psa", bufs=3, space="PSUM"))
            pst = pa_ctx.enter_context(tc.tile_pool(name="pst", bufs=2, space="PSUM"))

            whi_sb = wp.tile([P, 16, 512], F8, tag="whi")
            wlo_sb = wp.tile([P, 16, 512], F8, tag="wlo")
            nc.sync.dma_start(whi_sb[:, 0:8], whi[:, 0:8])
            nc.sync.dma_start(wlo_sb[:, 0:8], wlo[:, 0:8])
            nc.sync.dma_start(whi_sb[:, 8:16], whi[:, 8:16])
            nc.sync.dma_start(wlo_sb[:, 8:16], wlo[:, 8:16])

            for ts in range(4):          # 512-token superblocks
                x_sb = xp.tile([P, 16, 2, 512], F8, tag="x")
                ab_sb = xp.tile([P, 4, 4, 3, 64], F16, tag="ab")
                # split X panel DMA so the first matmuls start early
                nc.sync.dma_start(x_sb[:, 0:4], xhl[:, ts, 0:4])
                nc.sync.dma_start(ab_sb[:], ab[ts * 512:(ts + 1) * 512]
                                  .rearrange("(a p) c h d -> p a c h d", p=P))
                nc.sync.dma_start(x_sb[:, 4:10], xhl[:, ts, 4:10])
                nc.sync.dma_start(x_sb[:, 10:16], xhl[:, ts, 10:16])

                for tg in range(2):      # groups of 2 t-tiles (256 tokens)
                    qk16 = pa.tile([P, 2, 3, P], F16, tag="qk16")
                    sq = pa.tile([P, 2, 3, P], F16, tag="sq")
                    qkn = pa.tile([P, 2, 3, P], F16, tag="qkn")
                    m1 = pa.tile([P, 2, 3, 64], F16, tag="m1")
                    m2 = pa.tile([P, 2, 3, 64], F16, tag="m2")
                    ss = pa.tile([P, 2, 3], F32, tag="ss")
                    sr = pa.tile([P, 2, 3], F32, tag="sr")
                    si = pa.tile([P, 2, 3], F32, tag="si")

                    for ttl in range(2):
                        tt = ts * 4 + tg * 2 + ttl
                        tsl = slice((tg * 2 + ttl) * P, (tg * 2 + ttl + 1) * P)
                        ps_a = psa.tile([P, 512], F32, tag="psa")
                        # (X_hi + X_lo) @ W_hi : hi/lo as DoubleRow halves
                        for c in range(16):
                            nc.tensor.matmul(
                                ps_a[:],
                                x_sb[:, c, :, tsl],
                                whi_sb[:, c, None, :].to_broadcast((P, 2, 512)),
                                start=(c == 0),
                                stop=False,
                                perf_mode=DR,
                            )
                        # X_hi @ W_lo : chunk pairs as DoubleRow halves
                        for cp in range(8):
                            nc.tensor.matmul(
                                ps_a[:],
                                x_sb[:, 2 * cp:2 * cp + 2, 0, tsl],
                                wlo_sb[:, 2 * cp:2 * cp + 2, :],
                                start=False,
                                stop=(cp == 7),
                                perf_mode=DR,
                            )
                        # v: psum(64x) -> fp8 at 32x
                        nc.scalar.activation(
                            v8[:, tt, :], ps_a[:, 384:512], AF.Copy, scale=0.5
                        )
                        # q/k raw copy (64x scale) for rope + stats
                        nc.scalar.copy(qk16[:, ttl], ps_a[:, 0:384])
                        # sq = (0.25*x)^2 = 256*q_u^2  (f16-safe)
                        nc.scalar.activation(
                            sq[:, ttl], ps_a[:, 0:384], AF.Square, scale=0.25
                        )
                        nc.vector.tensor_reduce(
                            ss[:, ttl], sq[:, ttl], axis=mybir.AxisListType.X,
                            op=ALU.add,
                        )

                    # rms_scaled(64x) = sqrt(ss*0.125 + 4096*eps)
                    nc.scalar.activation(
                        sr[:], ss[:], AF.Sqrt, scale=0.125, bias=eps_sb[:]
                    )
                    nc.vector.reciprocal(si[:], sr[:])

                    # neox rope in fp16 (DVE 2x mode): tables fold norm weight
                    abg = ab_sb[:, 2 * tg:2 * tg + 2]    # [P, 2, 4, 3, 64]
                    x1 = qk16[:, :, :, 0:64]
                    x2 = qk16[:, :, :, 64:128]
                    nc.vector.tensor_mul(m1[:], x1, abg[:, :, 0])
                    nc.vector.tensor_mul(m2[:], x2, abg[:, :, 1])
                    nc.vector.tensor_sub(qkn[:, :, :, 0:64], m1[:], m2[:])
                    nc.vector.tensor_mul(m1[:], x2, abg[:, :, 2])
                    nc.vector.tensor_mul(m2[:], x1, abg[:, :, 3])
                    nc.vector.tensor_add(qkn[:, :, :, 64:128], m1[:], m2[:])
                    # apply 1/rms (broadcast over d)
                    nc.vector.tensor_mul(
                        qkn[:], qkn[:],
                        si[:, :, :, None].to_broadcast((P, 2, 3, P)),
                    )

                    # transpose to [d, t] (fp16 PE transposes, fp16 psum)
                    for ttl in range(2):
                        tt = ts * 4 + tg * 2 + ttl
                        ps_t = pst.tile([P, 3, P], F16, tag="pst")
                        for h in range(3):
                            nc.tensor.transpose(
                                ps_t[:, h, :], qkn[:, ttl, h, :], ident_sb[:]
                            )
                        nc.vector.tensor_copy(
                            qkT[:, :, tt * P:(tt + 1) * P], ps_t[:]
                        )

        # ---------------- Phase B: attention + o_proj -----------------------
        with ExitStack() as pb_ctx:
            expp = pb_ctx.enter_context(tc.tile_pool(name="expp", bufs=3))
            nrm = pb_ctx.enter_context(tc.tile_pool(name="nrm", bufs=2))
            outp = pb_ctx.enter_context(tc.tile_pool(name="outp", bufs=2))
            pss = pb_ctx.enter_context(tc.tile_pool(name="pss", bufs=2, space="PSUM"))
            psat = pb_ctx.enter_context(tc.tile_pool(name="psat", bufs=1, space="PSUM"))
            psse = pb_ctx.enter_context(tc.tile_pool(name="psse", bufs=1, space="PSUM"))
            pso = pb_ctx.enter_context(tc.tile_pool(name="pso", bufs=2, space="PSUM"))

            o_stage = {}
            o_count = 0

            def emit_oproj_unit(tt, hb):
                nonlocal o_count
                if hb == 0:
                    o_stage[tt] = outp.tile(
                        [P, 4, 512], F16, tag="ostg", name=f"ostg_{tt}"
                    )
                ps_o = pso.tile([P, 512], F32, tag="o")
                for hh in range(2):
                    nc.tensor.matmul(
                        ps_o[:],
                        attn16[:, hh, tt * P:(tt + 1) * P],
                        wo_sb[:, hh, hb * 512:(hb + 1) * 512],
                        start=(hh == 0),
                        stop=(hh == 1),
                    )
                dst = o_stage[tt][:, hb, :]
                r = o_count % 3
                o_count += 1
                if r == 0:
                    nc.scalar.copy(dst, ps_o[:])
                elif r == 1:
                    nc.vector.tensor_copy(dst, ps_o[:])
                else:
                    nc.gpsimd.tensor_copy(dst, ps_o[:])
                if hb == 3:
                    nc.sync.dma_start(
                        out[tt * P:(tt + 1) * P, :],
                        o_stage[tt][:].rearrange("p a c -> p (a c)"),
                    )

            oproj_queue = []

            def queue_oproj(tbo):
                for ttl in range(4):
                    for hb in range(4):
                        oproj_queue.append((4 * tbo + ttl, hb))

            for tb in range(N_TB):
                q0 = tb * 512
                npair = 2 * (tb + 1)
                for qh in range(2):
                    ps_at = psat.tile([P, 512], F32, tag="at")
                    ps_se = psse.tile([1, 512], F32, tag="se")

                    # pair order: diagonal pairs first, then off-diagonal
                    pair_order = [2 * tb, 2 * tb + 1] + list(range(2 * tb))

                    def emit_avse(jp, pi, o):
                        es = es_tiles[jp]
                        nc.tensor.matmul(
                            ps_at[:, o:512],
                            v8[:, 2 * jp:2 * jp + 2, :],
                            es[:, :, o:512],
                            start=(pi == 0),
                            stop=(pi == npair - 1),
                            perf_mode=DR,
                            skip_group_check=True,
                        )
                        nc.tensor.matmul(
                            ps_se[:, o:512],
                            ones_sb[:],
                            es[:, :, o:512],
                            start=(pi == 0),
                            stop=(pi == npair - 1),
                            perf_mode=DR,
                            skip_group_check=True,
                        )

                    es_tiles = {}
                    pending = []
                    for pi, jp in enumerate(pair_order):
                        off = jp - 2 * tb       # 0,1 for diagonal pairs
                        es = expp.tile([P, 2, 512], F8, tag="es")
                        es_tiles[jp] = es
                        ps_p = pss.tile([P, 2, 512], F32, tag="sp")
                        if 0 <= off < 2:
                            o = 256 * off
                            # zero the never-written leading es regions
                            if off == 0:
                                nc.gpsimd.memset(es[:, 1, 0:128], 0.0)
                            else:
                                nc.gpsimd.memset(es[:, 0, 0:256], 0.0)
                                nc.gpsimd.memset(es[:, 1, 0:384], 0.0)
                            for i in range(2):
                                oi = o + 128 * i
                                sb = 4 * tb + 2 * off + i
                                nc.tensor.matmul(
                                    ps_p[:, i, oi:512],
                                    qkT[:, 2, sb * P:(sb + 1) * P],
                                    qkT[:, qh, q0 + oi:q0 + 512],
                                    start=True,
                                    stop=False,
                                )
                                # causal triangle via PE: += maskT^T @ I
                                nc.tensor.matmul(
                                    ps_p[:, i, oi:oi + 128],
                                    maskt_sb[:],
                                    ident_sb[:],
                                    start=False,
                                    stop=True,
                                )
                                nc.scalar.activation(
                                    es[:, i, oi:512], ps_p[:, i, oi:512],
                                    AF.Exp, scale=SCALE, bias=ebias_sb[:],
                                )
                            if tb == 0 and off == 0:
                                # t=0 row: single-entry softmax, make exact
                                nc.gpsimd.memset(es[0:1, 0, 0:1], 1.0)
                            o_region = o
                        else:
                            for i in range(2):
                                sb = 2 * jp + i
                                nc.tensor.matmul(
                                    ps_p[:, i, :],
                                    qkT[:, 2, sb * P:(sb + 1) * P],
                                    qkT[:, qh, q0:q0 + 512],
                                    start=True,
                                    stop=True,
                                )
                            nc.scalar.activation(
                                es[:], ps_p[:], AF.Exp, scale=SCALE,
                                bias=ebias_sb[:],
                            )
                            o_region = 0
                        pending.append((jp, pi, o_region))
                        if len(pending) > 1:
                            emit_avse(*pending.pop(0))
                        if oproj_queue and (pi % 2 == 0 or len(oproj_queue) > 8):
                            emit_oproj_unit(*oproj_queue.pop(0))
                    for item in pending:
                        emit_avse(*item)

                    # softmax denominator -> broadcast -> normalize (fp16 out)
                    inv_sb = nrm.tile([1, 512], F32, tag="inv")
                    nc.vector.reciprocal(inv_sb[:], ps_se[:])
                    bc = nrm.tile([P, 512], F32, tag="bc")
                    nc.gpsimd.partition_broadcast(bc[:], inv_sb[0:1, :])
                    nc.vector.tensor_mul(
                        attn16[:, qh, q0:q0 + 512], ps_at[:], bc[:]
                    )
                while oproj_queue:
                    emit_oproj_unit(*oproj_queue.pop(0))
                queue_oproj(tb)
            while oproj_queue:
                emit_oproj_unit(*oproj_queue.pop(0))

    nc.compile()
    return nc


_NC_CACHE = {}


def _get_nc():
    if "nc" not in _NC_CACHE:
        _NC_CACHE["nc"] = _build_nc()
    return _NC_CACHE["nc"]


def kernel(positions, hidden_states, Wqkv, Wo, q_norm_w, k_norm_w):
    positions = np.asarray(positions)
    out_dtype = np.asarray(hidden_states).dtype
    hs = np.asarray(hidden_states, dtype=np.float32)
    Wqkv = np.asarray(Wqkv, dtype=np.float32)
    Wo = np.asarray(Wo, dtype=np.float32)
    qw = np.asarray(q_norm_w, dtype=np.float32)
    kw = np.asarray(k_norm_w, dtype=np.float32)

    # ----- host-side input prep -----
    xt = np.ascontiguousarray(hs.T)                       # [H, T]
    xhi = xt.astype(NP_F8)
    xlo = (xt - xhi.astype(np.float32)).astype(NP_F8)
    # [p, ts, c, l, t']
    xhl = np.empty((P, 4, 16, 2, 512), dtype=NP_F8)
    for l, arr in enumerate((xhi, xlo)):
        r = arr.reshape(16, P, 4, 512)                    # [c, p, ts, t']
        xhl[:, :, :, l, :] = r.transpose(1, 2, 0, 3)

    inv_freq = (1.0 / (THETA ** (np.arange(0, HD, 2, dtype=np.float32) / HD)))
    freqs = positions.astype(np.float32)[:, None] * inv_freq[None, :]
    cos = np.cos(freqs).astype(np.float32)
    sin = np.sin(freqs).astype(np.float32)

    def ab_tables(wvec):
        a1 = cos * wvec[None, :64]
        b1 = sin * wvec[None, 64:]
        a2 = cos * wvec[None, 64:]
        b2 = sin * wvec[None, :64]
        return np.stack([a1, b1, a2, b2], axis=1)         # [T, 4, 64]

    abq = ab_tables(qw)
    abk = ab_tables(kw)
    ab = np.ascontiguousarray(
        np.stack([abq, abq, abk], axis=2), dtype=np.float16
    )                                                     # [T, 4, 3, 64]

    # causal triangle block: maskT[c, p] = NEG if c < p else 0
    c_i = np.arange(P)
    maskt = np.where(c_i[:, None] < c_i[None, :], NEG, 0.0).astype(np.float16)
    ident = np.eye(P, dtype=np.float16)
    ones = np.full((P, 2), 32.0, dtype=NP_F8)

    q_size = N_HEADS * HD
    kv_size = N_KV * HD
    in_maps = []
    for j in range(N_CORES):
        qs = slice(2 * j * HD, (2 * j + 2) * HD)
        ks = slice(q_size + j * HD, q_size + (j + 1) * HD)
        vs = slice(q_size + kv_size + j * HD, q_size + kv_size + (j + 1) * HD)
        wj = np.concatenate(
            [Wqkv[:, qs], Wqkv[:, ks], Wqkv[:, vs]], axis=1
        ) * 64.0                                          # [H, 512]
        whi_f = wj.astype(NP_F8)
        wlo_f = (wj - whi_f.astype(np.float32)).astype(NP_F8)
        whi = np.ascontiguousarray(
            whi_f.reshape(16, P, 512).transpose(1, 0, 2))
        wlo = np.ascontiguousarray(
            wlo_f.reshape(16, P, 512).transpose(1, 0, 2))
        woj = np.ascontiguousarray(Wo[qs, :], dtype=np.float16)
        in_maps.append(
            {
                "xhl": xhl,
                "whi": whi,
                "wlo": wlo,
                "wo": woj,
                "ab": ab,
                "maskt": maskt,
                "ident": ident,
                "ones": ones,
            }
        )

    nc = _get_nc()
    res = run_bass_kernel_spmd(nc, in_maps, core_ids=list(range(N_CORES)))

    acc = res.results[0]["out"].astype(np.float32)
    for j in range(1, N_CORES):
        acc += res.results[j]["out"].astype(np.float32)
    return acc.astype(out_dtype, copy=False)
